# revision 5
# baseline (speedup 1.0000x reference)
"""Multi-head attention Trainium2 kernel (B=4, N=2048, D=1024, H=16).

Sharding: 8 cores = 4 batches x 2 head-groups (8 heads each), zero
collectives. Each core:
  - all projections in fp16, interleaved into the attention pipeline so
    the scalar engine (exp) starts ~10us in and stays saturated:
    x arrives as 512-column slices; the first head-pair's k-projection and
    q-projection run as the slices land, its S matmuls follow immediately,
    and the v-projection row-blocks are emitted just-in-time before the
    PV matmul that consumes them
  - q,k kept transposed [feat, seq]; v row-layout, augmented with a ones
    column so the PV matmul emits the softmax denominator for free
  - attention per head-pair x 512-query chunk: S matmuls packed two heads
    per pass via disjoint PE row groups into one [128,1024] PSUM tile,
    one wide exp on ACT (scale=1/8, fp16 out), PV accumulation with
    128-column stationary windows (fast-weight-load path)
  - software pipeline: each unit's S phase is split 8/8 around the
    previous unit's PV loop (exp pool holds ~24 tiles) so the scalar
    engine never starves; projection/out-projection/normalization work
    fills the PE slack inside the ACT-paced PV loops
  - normalization: one 64-wide ones matmul per head broadcasts the fp16
    denominator across partitions, one reciprocal_approx_fast and one
    multiply produce the normalized [128, 512] fp16 tile per unit
  - out-projection partial [1024,2048] per chunk, evacuated via DVE
Host sums the two head-group partials per batch and adds bias.
"""
from collections import deque
from contextlib import ExitStack

import numpy as np

import concourse.mybir as mybir
import concourse.tile as tile
from concourse import bacc
from concourse.bass_utils import run_bass_kernel_spmd

F32 = mybir.dt.float32
F16 = mybir.dt.float16

P = 128
N = 2048         # sequence length
DI = 1024        # model dim
NH = 8           # heads per core
HD = 64          # head dim
NPAIR = 4        # head pairs per core
KT = 8           # contraction tiles for projections
CH = 512         # query chunk width
NCHUNK = 4       # chunks per sequence
MT = 16          # key tiles (m) per sequence
ET = 8           # output-feature blocks
SCALE = HD ** -0.5
VW = HD + 1      # v columns per head incl. denominator ones-column
VFLAT = MT * NH * VW

_NC_CACHE = None


def _build():
    nc = bacc.Bacc("TRN2", target_bir_lowering=False, debug=False)

    xT = nc.dram_tensor("xT", [DI, N], F16, kind="ExternalInput").ap()
    wqkA = nc.dram_tensor("wqkA", [8, P, KT, P], F16, kind="ExternalInput").ap()
    wvA = nc.dram_tensor("wvA", [P, KT, 512], F16, kind="ExternalInput").ap()
    woT = nc.dram_tensor("woT", [512, DI], F16, kind="ExternalInput").ap()
    cstd = nc.dram_tensor("cst", [P, 129], F16, kind="ExternalInput").ap()
    outT = nc.dram_tensor("outT", [DI, N], F32, kind="ExternalOutput").ap()

    xT_r = xT.rearrange("(k p) n -> k p n", p=P)        # [8, 128, 2048]
    woT_r = woT.rearrange("(k p) e -> k p e", p=P)      # [4, 128, 1024]
    outT_r = outT.rearrange("(e p) n -> e p n", p=P)    # [8, 128, 2048]

    with tile.TileContext(nc) as tc, ExitStack() as persist:
        qk_pool = persist.enter_context(tc.tile_pool(name="qkp", bufs=8))
        va_pool = persist.enter_context(tc.tile_pool(name="vap", bufs=1))
        misc = persist.enter_context(tc.tile_pool(name="misc", bufs=1))
        wqk_pool = persist.enter_context(tc.tile_pool(name="wqk", bufs=2))
        xt_pool = persist.enter_context(tc.tile_pool(name="xt", bufs=8))
        wv_pool = persist.enter_context(tc.tile_pool(name="wv", bufs=1))
        wq_pool = persist.enter_context(tc.tile_pool(name="wq", bufs=4))
        wo_pool = persist.enter_context(tc.tile_pool(name="wo", bufs=4))
        exp_pool = persist.enter_context(tc.tile_pool(name="expp", bufs=26))
        ot_pool = persist.enter_context(tc.tile_pool(name="ot", bufs=8))
        osb_pool = persist.enter_context(tc.tile_pool(name="osb", bufs=4))
        stage_pool = persist.enter_context(tc.tile_pool(name="stg", bufs=3))
        den_pool = persist.enter_context(tc.tile_pool(name="den", bufs=8))
        rbc_pool = persist.enter_context(tc.tile_pool(name="rbc", bufs=2))
        sps_pool = persist.enter_context(
            tc.tile_pool(name="sps", bufs=2, space="PSUM"))
        oaug_pool = persist.enter_context(
            tc.tile_pool(name="oaug", bufs=2, space="PSUM"))
        aux_pool = persist.enter_context(
            tc.tile_pool(name="aux", bufs=2, space="PSUM"))

        # --- input DMAs, ordered so the pipeline lights up ASAP: the
        # first k-projection group needs wqk[4] + the chunk-0 columns of
        # every x k-tile; q needs wq0; v needs wv.
        cst = misc.tile([P, 129], F16)
        nc.sync.dma_start(cst[:], cstd[:])
        wqk_first = wqk_pool.tile([P, KT, P], F16, tag="wqk")
        nc.sync.dma_start(wqk_first[:], wqkA[4])
        xt = [xt_pool.tile([P, N], F16, name=f"xt{k}", tag="xt")
              for k in range(KT)]
        wq = [wq_pool.tile([P, KT, P], F16, name=f"wq{f}", tag="wq")
              for f in range(4)]
        for k in range(KT):
            nc.sync.dma_start(xt[k][:, 0:CH], xT_r[k][:, 0:CH])
        nc.sync.dma_start(wq[0][:], wqkA[0])
        wv = wv_pool.tile([P, KT, 512], F16)
        nc.sync.dma_start(wv[:], wvA[:])
        for cc in range(1, NCHUNK):
            csl = slice(cc * CH, (cc + 1) * CH)
            for k in range(KT):
                nc.sync.dma_start(xt[k][:, csl], xT_r[k][:, csl])
            if cc < 4:
                nc.sync.dma_start(wq[cc][:], wqkA[cc])
        wo = [wo_pool.tile([P, DI], F16, name=f"wo{kk}", tag="wo")
              for kk in range(NPAIR)]
        for kk in range(NPAIR):
            nc.sync.dma_start(wo[kk][:], woT_r[kk])

        qkT = [qk_pool.tile([P, N], F16, name=f"qkT{t}", tag="qkT")
               for t in range(8)]
        va_t = va_pool.tile([P, VFLAT + 64], F16)
        nc.vector.memset(va_t[:, VFLAT:VFLAT + 64], 0.0)
        v_aug = va_t[:, 0:VFLAT].rearrange("p (m h d) -> p m h d", h=NH, d=VW)
        nc.vector.tensor_copy(v_aug[:, :, :, HD:HD + 1],
                              cst[:, 0:1].to_broadcast((P, MT, NH, 1)))

        wqk_tiles = {4: wqk_first}

        # ---- emission helpers -------------------------------------------
        def emit_kproj_chunk(p, cc):
            f = 4 + p
            if f not in wqk_tiles:
                t = wqk_pool.tile([P, KT, P], F16, tag="wqk")
                nc.sync.dma_start(t[:], wqkA[f])
                wqk_tiles[f] = t
            wqk_f = wqk_tiles[f]
            csl = slice(cc * CH, (cc + 1) * CH)
            ps = aux_pool.tile([P, CH], F32, tag="aux", name=f"kp_{p}_{cc}")
            for k in range(KT):
                nc.tensor.matmul(ps[:], wqk_f[:, k, :], xt[k][:, csl],
                                 start=(k == 0), stop=(k == KT - 1))
            nc.vector.tensor_copy(qkT[4 + p][:, csl], ps[:])

        def emit_qproj(c, p):
            csl = slice(c * CH, (c + 1) * CH)
            ps = aux_pool.tile([P, CH], F32, tag="aux", name=f"qp_{c}_{p}")
            for k in range(KT):
                nc.tensor.matmul(ps[:], wq[p][:, k, :], xt[k][:, csl],
                                 start=(k == 0), stop=(k == KT - 1))
            nc.vector.tensor_copy(qkT[p][:, csl], ps[:])

        def emit_vproj(r):
            ps = aux_pool.tile([P, CH], F32, tag="aux", name=f"vp_{r}")
            for k in range(KT):
                nc.tensor.matmul(ps[:], xt[k][:, r * P:(r + 1) * P],
                                 wv[:, k, :],
                                 start=(k == 0), stop=(k == KT - 1))
            nc.vector.tensor_copy(v_aug[:, r, :, 0:HD],
                                  ps.rearrange("p (h d) -> p h d", d=HD))

        exp_map = {}   # (c, p) -> list of expP tiles

        def emit_S_pairs(c, p, ms):
            csl = slice(c * CH, (c + 1) * CH)
            qA = qkT[p][0:HD, csl]
            qB = qkT[p][HD:P, csl]
            kTl = qkT[4 + p]
            lst = exp_map.setdefault((c, p), [None] * MT)
            for m in ms:
                msl = slice(m * P, (m + 1) * P)
                s_ps = sps_pool.tile([P, 2 * CH], F32, tag="sps",
                                     name=f"sps_{c}_{p}_{m}")
                nc.tensor.matmul(s_ps[:, 0:CH], kTl[0:HD, msl], qA,
                                 start=True, stop=True)
                nc.tensor.matmul(s_ps[:, CH:2 * CH], kTl[HD:P, msl], qB,
                                 start=True, stop=True)
                expP = exp_pool.tile([P, 2 * CH], F16, tag="expp",
                                     name=f"expP_{c}_{p}_{m}")
                nc.scalar.activation(expP[:], s_ps[:],
                                     mybir.ActivationFunctionType.Exp,
                                     scale=SCALE)
                lst[m] = expP

        def emit_PV(c, p, fill):
            # fill: dict slot -> list of thunks emitted before that PV pair
            oaugA = oaug_pool.tile([P, CH], F32, tag="oaug",
                                   name=f"oaugA_{c}_{p}")
            oaugB = oaug_pool.tile([P, CH], F32, tag="oaug",
                                   name=f"oaugB_{c}_{p}")
            expPs = exp_map.pop((c, p))
            for m in range(MT):
                for th in fill.get(m, ()):
                    th()
                vbase = (m * NH + 2 * p) * VW
                nc.tensor.matmul(oaugA[:, :], va_t[:, vbase:vbase + P],
                                 expPs[m][:, 0:CH],
                                 start=(m == 0), stop=(m == MT - 1))
                vbase = (m * NH + 2 * p + 1) * VW
                nc.tensor.matmul(oaugB[:, :], va_t[:, vbase:vbase + P],
                                 expPs[m][:, CH:2 * CH],
                                 start=(m == 0), stop=(m == MT - 1))
            # evacuate numerators + denominators (DVE only)
            o_sb = osb_pool.tile([P, CH], F32, tag="osb", name=f"osb_{c}_{p}")
            denA = den_pool.tile([1, CH], F16, tag="den", name=f"denA_{c}_{p}")
            denB = den_pool.tile([1, CH], F16, tag="den", name=f"denB_{c}_{p}")
            nc.vector.tensor_copy(o_sb[0:HD, :], oaugA[0:HD, :])
            nc.vector.tensor_copy(o_sb[HD:P, :], oaugB[0:HD, :])
            with nc.allow_low_precision(reason="softmax denom fp16"):
                nc.vector.tensor_copy(denA[:], oaugA[HD:HD + 1, :])
                nc.vector.tensor_copy(denB[:], oaugB[HD:HD + 1, :])
            return (c, p, o_sb, denA, denB)

        ot_map = {}

        def emit_norm(unit):
            c, p, o_sb, denA, denB = unit
            bc = aux_pool.tile([P, CH], F32, tag="aux", name=f"bc_{c}_{p}")
            nc.tensor.matmul(bc[0:HD, :], cst[0:1, 1:65], denA[:],
                             start=True, stop=True)
            nc.tensor.matmul(bc[HD:P, :], cst[0:1, 1:65], denB[:],
                             start=True, stop=True)
            rbc = rbc_pool.tile([P, CH], F32, tag="rbc", name=f"rbc_{c}_{p}")
            nc.vector.reciprocal_approx_fast(out=rbc[:], in_=bc[:])
            ot_p = ot_pool.tile([P, CH], F16, name=f"ot_{c}_{p}", tag="ot")
            nc.vector.tensor_tensor(ot_p[:], o_sb[:], rbc[:],
                                    mybir.AluOpType.mult)
            ot_map[(c, p)] = ot_p

        def emit_outproj(c):
            csl = slice(c * CH, (c + 1) * CH)
            for e in range(ET):
                pso = aux_pool.tile([P, CH], F32, tag="aux",
                                    name=f"pso_{c}_{e}")
                for p in range(NPAIR):
                    nc.tensor.matmul(pso[:], wo[p][:, e * P:(e + 1) * P],
                                     ot_map[(c, p)][:],
                                     start=(p == 0), stop=(p == NPAIR - 1))
                st = stage_pool.tile([P, CH], F32, tag="stg",
                                     name=f"st_{c}_{e}")
                nc.vector.tensor_copy(st[:], pso[:])
                nc.sync.dma_start(outT_r[e][:, csl], st[:])

        # ---- the pipeline -----------------------------------------------
        units = [(c, p) for c in range(NCHUNK) for p in range(NPAIR)]

        # prologue: unit (0,0) S phase, k-projection interleaved per chunk
        for cc in range(NCHUNK):
            emit_kproj_chunk(0, cc)
            if cc == 0:
                emit_qproj(0, 0)
            emit_S_pairs(0, 0, range(4 * cc, 4 * cc + 4))
        # hoist first half of unit (0,1)'s S phase
        for cc in range(NCHUNK):
            emit_kproj_chunk(1, cc)
        emit_qproj(0, 1)
        emit_S_pairs(0, 1, range(0, 8))

        pend_norm = deque()
        normed = {c: 0 for c in range(NCHUNK)}
        outproj_done = set()

        for i, (c, p) in enumerate(units):
            nxt = units[i + 1] if i + 1 < len(units) else None
            nxt2 = units[i + 2] if i + 2 < len(units) else None

            fill = {}
            if (c, p) == (0, 0):
                # v-projection row-blocks, just-in-time for PV
                for m in range(MT):
                    fill.setdefault(m, []).append(
                        lambda r=m: emit_vproj(r))
            if nxt is not None:
                # second half of the next unit's S phase
                for j, m in enumerate(range(8, MT)):
                    fill.setdefault(j * 2, []).append(
                        lambda u=nxt, mm=m: emit_S_pairs(u[0], u[1], [mm]))

            unit = emit_PV(c, p, fill)
            pend_norm.append(unit)

            # post-block: norm (lagged), out-projection, next-next prologue
            if len(pend_norm) > 1:
                u = pend_norm.popleft()
                emit_norm(u)
                normed[u[0]] += 1
            for cc in range(NCHUNK):
                if (normed[cc] == NPAIR and cc not in outproj_done):
                    emit_outproj(cc)
                    outproj_done.add(cc)
            if nxt2 is not None:
                c2, p2 = nxt2
                if c2 == 0:
                    for cc in range(NCHUNK):
                        emit_kproj_chunk(p2, cc)
                emit_qproj(c2, p2)
                emit_S_pairs(c2, p2, range(0, 8))

        # tail: drain norms + remaining out-projections
        while pend_norm:
            u = pend_norm.popleft()
            emit_norm(u)
            normed[u[0]] += 1
        for cc in range(NCHUNK):
            if normed[cc] == NPAIR and cc not in outproj_done:
                emit_outproj(cc)
                outproj_done.add(cc)

    nc.compile()
    return nc


def _get_nc():
    global _NC_CACHE
    if _NC_CACHE is None:
        _NC_CACHE = _build()
    return _NC_CACHE


def _make_in_maps(x, w_qkv, w_out):
    cst = np.zeros((P, 129), dtype=np.float16)
    cst[:, 0] = 1.0
    cst[0, 1:65] = 1.0
    cst[1, 65:129] = 1.0
    per_g = []
    for g in range(2):
        qk_g = np.concatenate([w_qkv[g * 512:(g + 1) * 512],
                               w_qkv[DI + g * 512:DI + (g + 1) * 512]], axis=0)
        wqkT = np.ascontiguousarray(qk_g.T)               # [1024 d, 1024 f]
        wqkA = np.ascontiguousarray(
            wqkT.reshape(KT, P, 8, P).transpose(2, 1, 0, 3).astype(np.float16))
        v_g = w_qkv[2 * DI + g * 512:2 * DI + (g + 1) * 512]
        wvT = np.ascontiguousarray(v_g.T)                 # [1024 d, 512 f]
        wvA = np.ascontiguousarray(
            wvT.reshape(KT, P, 512).transpose(1, 0, 2).astype(np.float16))
        woTg = np.ascontiguousarray(
            w_out[:, g * 512:(g + 1) * 512].T.astype(np.float16))
        per_g.append((wqkA, wvA, woTg))

    in_maps = []
    for c in range(8):
        b, g = c // 2, c % 2
        wqkA, wvA, woTg = per_g[g]
        in_maps.append({
            "xT": np.ascontiguousarray(x[b].T.astype(np.float16)),
            "wqkA": wqkA,
            "wvA": wvA,
            "woT": woTg,
            "cst": cst,
        })
    return in_maps


def kernel(x, w_qkv, w_out, b_out):
    x = np.asarray(x, dtype=np.float32)
    w_qkv = np.asarray(w_qkv, dtype=np.float32)
    w_out = np.asarray(w_out, dtype=np.float32)
    b_out = np.asarray(b_out, dtype=np.float32)
    B = x.shape[0]

    in_maps = _make_in_maps(x, w_qkv, w_out)
    nc = _get_nc()
    res = run_bass_kernel_spmd(nc, in_maps, core_ids=list(range(8)))
    parts = [r["outT"] for r in res.results]
    out = np.empty((B, N, DI), dtype=np.float32)
    for b in range(B):
        out[b] = (parts[2 * b] + parts[2 * b + 1]).T + b_out
    return out


# revision 8
# speedup vs baseline: 1.1492x; 1.1492x over previous
"""Multi-head attention Trainium2 kernel (B=4, N=2048, D=1024, H=16).

Sharding: 8 cores = 4 batches x 2 head-groups (8 heads each), zero
collectives. Each core:
  - all projections in fp16, interleaved into the attention pipeline so
    the scalar engine (exp) starts ~10us in and stays saturated:
    x arrives as 512-column slices; the first head-pair's k-projection and
    q-projection run as the slices land, its S matmuls follow immediately,
    and the v-projection row-blocks are emitted just-in-time before the
    PV matmul that consumes them
  - q,k kept transposed [feat, seq]; v row-layout, augmented with a ones
    column so the PV matmul emits the softmax denominator for free
  - attention per head-pair x 512-query chunk: S matmuls packed two heads
    per pass via disjoint PE row groups into one [128,1024] PSUM tile,
    one wide exp on ACT (scale=1/8, fp16 out), PV accumulation with
    128-column stationary windows (fast-weight-load path)
  - software pipeline: each unit's S phase is split 8/8 around the
    previous unit's PV loop (exp pool holds ~24 tiles) so the scalar
    engine never starves; projection/out-projection/normalization work
    fills the PE slack inside the ACT-paced PV loops
  - normalization: one 64-wide ones matmul per head broadcasts the fp16
    denominator across partitions, one reciprocal_approx_fast and one
    multiply produce the normalized [128, 512] fp16 tile per unit
  - out-projection partial [1024,2048] per chunk, evacuated via DVE
Host sums the two head-group partials per batch and adds bias.
"""
from collections import deque
from contextlib import ExitStack

import numpy as np

import concourse.mybir as mybir
import concourse.tile as tile
from concourse import bacc
from concourse.bass_utils import run_bass_kernel_spmd

F32 = mybir.dt.float32
F16 = mybir.dt.float16

P = 128
N = 2048         # sequence length
DI = 1024        # model dim
NH = 8           # heads per core
HD = 64          # head dim
NPAIR = 4        # head pairs per core
KT = 8           # contraction tiles for projections
CH = 512         # query chunk width
NCHUNK = 4       # chunks per sequence
MT = 16          # key tiles (m) per sequence
ET = 8           # output-feature blocks
SCALE = HD ** -0.5
VW = HD + 1      # v columns per head incl. denominator ones-column
VFLAT = MT * NH * VW

_NC_CACHE = None


def _build():
    nc = bacc.Bacc("TRN2", target_bir_lowering=False, debug=False)

    xT = nc.dram_tensor("xT", [DI, N], F16, kind="ExternalInput").ap()
    wqkA = nc.dram_tensor("wqkA", [8, P, KT, P], F16, kind="ExternalInput").ap()
    wvA = nc.dram_tensor("wvA", [P, KT, 512], F16, kind="ExternalInput").ap()
    woT = nc.dram_tensor("woT", [512, DI], F16, kind="ExternalInput").ap()
    cstd = nc.dram_tensor("cst", [P, 129], F16, kind="ExternalInput").ap()
    outT = nc.dram_tensor("outT", [DI, N], F32, kind="ExternalOutput").ap()

    xT_r = xT.rearrange("(k p) n -> k p n", p=P)        # [8, 128, 2048]
    woT_r = woT.rearrange("(k p) e -> k p e", p=P)      # [4, 128, 1024]
    outT_r = outT.rearrange("(e p) n -> e p n", p=P)    # [8, 128, 2048]

    with tile.TileContext(nc) as tc, ExitStack() as persist:
        qk_pool = persist.enter_context(tc.tile_pool(name="qkp", bufs=8))
        va_pool = persist.enter_context(tc.tile_pool(name="vap", bufs=1))
        misc = persist.enter_context(tc.tile_pool(name="misc", bufs=1))
        wqk_pool = persist.enter_context(tc.tile_pool(name="wqk", bufs=2))
        xt_pool = persist.enter_context(tc.tile_pool(name="xt", bufs=8))
        wv_pool = persist.enter_context(tc.tile_pool(name="wv", bufs=1))
        wq_pool = persist.enter_context(tc.tile_pool(name="wq", bufs=4))
        wo_pool = persist.enter_context(tc.tile_pool(name="wo", bufs=4))
        exp_pool = persist.enter_context(tc.tile_pool(name="expp", bufs=26))
        ot_pool = persist.enter_context(tc.tile_pool(name="ot", bufs=8))
        osb_pool = persist.enter_context(tc.tile_pool(name="osb", bufs=4))
        stage_pool = persist.enter_context(tc.tile_pool(name="stg", bufs=3))
        den_pool = persist.enter_context(tc.tile_pool(name="den", bufs=8))
        rbc_pool = persist.enter_context(tc.tile_pool(name="rbc", bufs=2))
        sps_pool = persist.enter_context(
            tc.tile_pool(name="sps", bufs=2, space="PSUM"))
        oaug_pool = persist.enter_context(
            tc.tile_pool(name="oaug", bufs=2, space="PSUM"))
        aux_pool = persist.enter_context(
            tc.tile_pool(name="aux", bufs=2, space="PSUM"))

        # --- input DMAs, ordered so the pipeline lights up ASAP: the
        # first k-projection group needs wqk[4] + the chunk-0 columns of
        # every x k-tile; q needs wq0; v needs wv.
        cst = misc.tile([P, 129], F16)
        nc.sync.dma_start(cst[:], cstd[:])
        wqk_first = wqk_pool.tile([P, KT, P], F16, tag="wqk")
        nc.sync.dma_start(wqk_first[:], wqkA[4])
        xt = [xt_pool.tile([P, N], F16, name=f"xt{k}", tag="xt")
              for k in range(KT)]
        wq = [wq_pool.tile([P, KT, P], F16, name=f"wq{f}", tag="wq")
              for f in range(4)]
        for k in range(KT):
            nc.sync.dma_start(xt[k][:, 0:CH], xT_r[k][:, 0:CH])
        nc.sync.dma_start(wq[0][:], wqkA[0])
        wv = wv_pool.tile([P, KT, 512], F16)
        nc.sync.dma_start(wv[:], wvA[:])
        for cc in range(1, NCHUNK):
            csl = slice(cc * CH, (cc + 1) * CH)
            for k in range(KT):
                nc.sync.dma_start(xt[k][:, csl], xT_r[k][:, csl])
            if cc < 4:
                nc.sync.dma_start(wq[cc][:], wqkA[cc])
        wo = [wo_pool.tile([P, DI], F16, name=f"wo{kk}", tag="wo")
              for kk in range(NPAIR)]
        for kk in range(NPAIR):
            nc.sync.dma_start(wo[kk][:], woT_r[kk])

        qkT = [qk_pool.tile([P, N], F16, name=f"qkT{t}", tag="qkT")
               for t in range(8)]
        va_t = va_pool.tile([P, VFLAT + 64], F16)
        nc.vector.memset(va_t[:, VFLAT:VFLAT + 64], 0.0)
        v_aug = va_t[:, 0:VFLAT].rearrange("p (m h d) -> p m h d", h=NH, d=VW)
        nc.vector.tensor_copy(v_aug[:, :, :, HD:HD + 1],
                              cst[:, 0:1].to_broadcast((P, MT, NH, 1)))

        wqk_tiles = {4: wqk_first}

        # ---- emission helpers -------------------------------------------
        def emit_kproj_chunk(p, cc):
            f = 4 + p
            if f not in wqk_tiles:
                t = wqk_pool.tile([P, KT, P], F16, tag="wqk")
                nc.sync.dma_start(t[:], wqkA[f])
                wqk_tiles[f] = t
            wqk_f = wqk_tiles[f]
            csl = slice(cc * CH, (cc + 1) * CH)
            ps = aux_pool.tile([P, CH], F32, tag="aux", name=f"kp_{p}_{cc}")
            for k in range(KT):
                nc.tensor.matmul(ps[:], wqk_f[:, k, :], xt[k][:, csl],
                                 start=(k == 0), stop=(k == KT - 1))
            nc.vector.tensor_copy(qkT[4 + p][:, csl], ps[:])

        def emit_qproj(c, p):
            csl = slice(c * CH, (c + 1) * CH)
            ps = aux_pool.tile([P, CH], F32, tag="aux", name=f"qp_{c}_{p}")
            for k in range(KT):
                nc.tensor.matmul(ps[:], wq[p][:, k, :], xt[k][:, csl],
                                 start=(k == 0), stop=(k == KT - 1))
            nc.vector.tensor_copy(qkT[p][:, csl], ps[:])

        def emit_vproj(r):
            ps = aux_pool.tile([P, CH], F32, tag="aux", name=f"vp_{r}")
            for k in range(KT):
                nc.tensor.matmul(ps[:], xt[k][:, r * P:(r + 1) * P],
                                 wv[:, k, :],
                                 start=(k == 0), stop=(k == KT - 1))
            nc.vector.tensor_copy(v_aug[:, r, :, 0:HD],
                                  ps.rearrange("p (h d) -> p h d", d=HD))

        exp_map = {}   # (c, p) -> list of expP tiles

        def emit_S_pairs(c, p, ms):
            csl = slice(c * CH, (c + 1) * CH)
            qA = qkT[p][0:HD, csl]
            qB = qkT[p][HD:P, csl]
            kTl = qkT[4 + p]
            lst = exp_map.setdefault((c, p), [None] * MT)
            for m in ms:
                msl = slice(m * P, (m + 1) * P)
                s_ps = sps_pool.tile([P, 2 * CH], F32, tag="sps",
                                     name=f"sps_{c}_{p}_{m}")
                nc.tensor.matmul(s_ps[:, 0:CH], kTl[0:HD, msl], qA,
                                 start=True, stop=True)
                nc.tensor.matmul(s_ps[:, CH:2 * CH], kTl[HD:P, msl], qB,
                                 start=True, stop=True)
                expP = exp_pool.tile([P, 2 * CH], F16, tag="expp",
                                     name=f"expP_{c}_{p}_{m}")
                nc.scalar.activation(expP[:], s_ps[:],
                                     mybir.ActivationFunctionType.Exp,
                                     scale=SCALE)
                lst[m] = expP

        def emit_PV(c, p, fill):
            # fill: dict slot -> list of thunks emitted before that PV matmul.
            # The two heads' PV accumulations run as separate contiguous
            # sweeps (A then B) so each group's weight loads pull ahead into
            # the background buffer instead of serializing on the group
            # switch; the A sweep is exp-paced, the B sweep runs dense.
            oaugA = oaug_pool.tile([P, CH], F32, tag="oaug",
                                   name=f"oaugA_{c}_{p}")
            oaugB = oaug_pool.tile([P, CH], F32, tag="oaug",
                                   name=f"oaugB_{c}_{p}")
            expPs = exp_map.pop((c, p))
            for m in range(MT):
                for th in fill.get(m, ()):
                    th()
                vbase = (m * NH + 2 * p) * VW
                nc.tensor.matmul(oaugA[:, :], va_t[:, vbase:vbase + P],
                                 expPs[m][:, 0:CH],
                                 start=(m == 0), stop=(m == MT - 1))
            for m in range(MT):
                vbase = (m * NH + 2 * p + 1) * VW
                nc.tensor.matmul(oaugB[:, :], va_t[:, vbase:vbase + P],
                                 expPs[m][:, CH:2 * CH],
                                 start=(m == 0), stop=(m == MT - 1))
            # evacuate numerators + denominators (DVE only)
            o_sb = osb_pool.tile([P, CH], F32, tag="osb", name=f"osb_{c}_{p}")
            denA = den_pool.tile([1, CH], F16, tag="den", name=f"denA_{c}_{p}")
            denB = den_pool.tile([1, CH], F16, tag="den", name=f"denB_{c}_{p}")
            nc.vector.tensor_copy(o_sb[0:HD, :], oaugA[0:HD, :])
            nc.vector.tensor_copy(o_sb[HD:P, :], oaugB[0:HD, :])
            with nc.allow_low_precision(reason="softmax denom fp16"):
                nc.vector.tensor_copy(denA[:], oaugA[HD:HD + 1, :])
                nc.vector.tensor_copy(denB[:], oaugB[HD:HD + 1, :])
            return (c, p, o_sb, denA, denB)

        ot_map = {}

        def emit_norm(unit):
            c, p, o_sb, denA, denB = unit
            bc = aux_pool.tile([P, CH], F32, tag="aux", name=f"bc_{c}_{p}")
            nc.tensor.matmul(bc[0:HD, :], cst[0:1, 1:65], denA[:],
                             start=True, stop=True)
            nc.tensor.matmul(bc[HD:P, :], cst[0:1, 1:65], denB[:],
                             start=True, stop=True)
            rbc = rbc_pool.tile([P, CH], F32, tag="rbc", name=f"rbc_{c}_{p}")
            nc.vector.reciprocal_approx_fast(out=rbc[:], in_=bc[:])
            ot_p = ot_pool.tile([P, CH], F16, name=f"ot_{c}_{p}", tag="ot")
            nc.vector.tensor_tensor(ot_p[:], o_sb[:], rbc[:],
                                    mybir.AluOpType.mult)
            ot_map[(c, p)] = ot_p

        def emit_outproj_e(c, e):
            csl = slice(c * CH, (c + 1) * CH)
            pso = aux_pool.tile([P, CH], F32, tag="aux",
                                name=f"pso_{c}_{e}")
            for p in range(NPAIR):
                nc.tensor.matmul(pso[:], wo[p][:, e * P:(e + 1) * P],
                                 ot_map[(c, p)][:],
                                 start=(p == 0), stop=(p == NPAIR - 1))
            st = stage_pool.tile([P, CH], F32, tag="stg",
                                 name=f"st_{c}_{e}")
            nc.vector.tensor_copy(st[:], pso[:])
            nc.sync.dma_start(outT_r[e][:, csl], st[:])

        # ---- the pipeline -----------------------------------------------
        units = [(c, p) for c in range(NCHUNK) for p in range(NPAIR)]

        # prologue: unit (0,0) S phase with k-projection per chunk and the
        # first half of the v-projection woven in (all DMA-covered)
        for cc in range(NCHUNK):
            emit_kproj_chunk(0, cc)
            if cc == 0:
                emit_qproj(0, 0)
            emit_S_pairs(0, 0, range(4 * cc, 4 * cc + 4))
            if cc < 2:
                for r in range(4 * cc, 4 * cc + 4):
                    emit_vproj(r)
        # hoist first half of unit (0,1)'s S phase
        for cc in range(NCHUNK):
            emit_kproj_chunk(1, cc)
        emit_qproj(0, 1)
        emit_S_pairs(0, 1, range(0, 8))

        pend_norm = deque()
        normed = {c: 0 for c in range(NCHUNK)}
        pend_outproj = deque()

        for i, (c, p) in enumerate(units):
            nxt = units[i + 1] if i + 1 < len(units) else None
            nxt2 = units[i + 2] if i + 2 < len(units) else None

            fill = {}
            if (c, p) == (0, 0):
                # remaining v-projection row-blocks, just-in-time for PV
                for m in range(8, MT):
                    fill.setdefault(m, []).append(lambda r=m: emit_vproj(r))
            if nxt is not None:
                # second half of the next unit's S phase
                for j, m in enumerate(range(8, MT)):
                    fill.setdefault(j * 2, []).append(
                        lambda u=nxt, mm=m: emit_S_pairs(u[0], u[1], [mm]))
            if nxt2 is not None and nxt2[0] == 0:
                # k-projection for the unit after next, spread across slots
                for j in range(NCHUNK):
                    fill.setdefault(2 * j + 1, []).append(
                        lambda p2=nxt2[1], cc=j: emit_kproj_chunk(p2, cc))
            # spread pending out-projection blocks across late slots
            for j in range(8, MT):
                if pend_outproj:
                    th = pend_outproj.popleft()
                    fill.setdefault(j, []).append(th)

            unit = emit_PV(c, p, fill)
            pend_norm.append(unit)

            # post-block: norm (lagged), then next-next unit's q + S half
            # (the S pairs keep the scalar engine fed)
            if len(pend_norm) > 1:
                u = pend_norm.popleft()
                emit_norm(u)
                normed[u[0]] += 1
                if normed[u[0]] == NPAIR:
                    cc = u[0]
                    for e in range(ET):
                        pend_outproj.append(
                            lambda c2=cc, ee=e: emit_outproj_e(c2, ee))
            if nxt2 is not None:
                c2, p2 = nxt2
                emit_qproj(c2, p2)
                emit_S_pairs(c2, p2, range(0, 8))

        # tail: drain norms + remaining out-projections
        while pend_norm:
            u = pend_norm.popleft()
            emit_norm(u)
            normed[u[0]] += 1
        for th in pend_outproj:
            th()
        for e in range(ET):
            emit_outproj_e(NCHUNK - 1, e)

    nc.compile()
    return nc


def _get_nc():
    global _NC_CACHE
    if _NC_CACHE is None:
        _NC_CACHE = _build()
    return _NC_CACHE


def _make_in_maps(x, w_qkv, w_out):
    cst = np.zeros((P, 129), dtype=np.float16)
    cst[:, 0] = 1.0
    cst[0, 1:65] = 1.0
    cst[1, 65:129] = 1.0
    per_g = []
    for g in range(2):
        qk_g = np.concatenate([w_qkv[g * 512:(g + 1) * 512],
                               w_qkv[DI + g * 512:DI + (g + 1) * 512]], axis=0)
        wqkT = np.ascontiguousarray(qk_g.T)               # [1024 d, 1024 f]
        wqkA = np.ascontiguousarray(
            wqkT.reshape(KT, P, 8, P).transpose(2, 1, 0, 3).astype(np.float16))
        v_g = w_qkv[2 * DI + g * 512:2 * DI + (g + 1) * 512]
        wvT = np.ascontiguousarray(v_g.T)                 # [1024 d, 512 f]
        wvA = np.ascontiguousarray(
            wvT.reshape(KT, P, 512).transpose(1, 0, 2).astype(np.float16))
        woTg = np.ascontiguousarray(
            w_out[:, g * 512:(g + 1) * 512].T.astype(np.float16))
        per_g.append((wqkA, wvA, woTg))

    in_maps = []
    for c in range(8):
        b, g = c // 2, c % 2
        wqkA, wvA, woTg = per_g[g]
        in_maps.append({
            "xT": np.ascontiguousarray(x[b].T.astype(np.float16)),
            "wqkA": wqkA,
            "wvA": wvA,
            "woT": woTg,
            "cst": cst,
        })
    return in_maps


def kernel(x, w_qkv, w_out, b_out):
    x = np.asarray(x, dtype=np.float32)
    w_qkv = np.asarray(w_qkv, dtype=np.float32)
    w_out = np.asarray(w_out, dtype=np.float32)
    b_out = np.asarray(b_out, dtype=np.float32)
    B = x.shape[0]

    in_maps = _make_in_maps(x, w_qkv, w_out)
    nc = _get_nc()
    res = run_bass_kernel_spmd(nc, in_maps, core_ids=list(range(8)))
    parts = [r["outT"] for r in res.results]
    out = np.empty((B, N, DI), dtype=np.float32)
    for b in range(B):
        out[b] = (parts[2 * b] + parts[2 * b + 1]).T + b_out
    return out


# revision 15
# speedup vs baseline: 1.1566x; 1.0064x over previous
"""Multi-head attention Trainium2 kernel (B=4, N=2048, D=1024, H=16).

Sharding: 8 cores = 4 batches x 2 head-groups (8 heads each), zero
collectives. Each core:
  - all projections in fp16, interleaved into the attention pipeline so
    the scalar engine (exp) starts ~10us in and stays saturated:
    x arrives as 512-column slices; the first head-pair's k-projection and
    q-projection run as the slices land, its S matmuls follow immediately,
    and the v-projection row-blocks are emitted just-in-time before the
    PV matmul that consumes them
  - q,k kept transposed [feat, seq]; v row-layout, augmented with a ones
    column so the PV matmul emits the softmax denominator for free
  - attention per head-pair x 512-query chunk: S matmuls packed two heads
    per pass via disjoint PE row groups into one [128,1024] PSUM tile,
    one wide exp on ACT (scale=1/8, fp16 out), PV accumulation with
    128-column stationary windows (fast-weight-load path)
  - software pipeline: each unit's S phase is split 8/8 around the
    previous unit's PV loop (exp pool holds ~24 tiles) so the scalar
    engine never starves; projection/out-projection/normalization work
    fills the PE slack inside the ACT-paced PV loops
  - normalization: one 64-wide ones matmul per head broadcasts the fp16
    denominator across partitions, one reciprocal_approx_fast and one
    multiply produce the normalized [128, 512] fp16 tile per unit
  - out-projection partial [1024,2048] per chunk, evacuated via DVE
Host sums the two head-group partials per batch and adds bias.
"""
from collections import deque
from contextlib import ExitStack

import numpy as np

import concourse.mybir as mybir
import concourse.tile as tile
from concourse import bacc
from concourse.bass_utils import run_bass_kernel_spmd

F32 = mybir.dt.float32
F16 = mybir.dt.float16

P = 128
N = 2048         # sequence length
DI = 1024        # model dim
NH = 8           # heads per core
HD = 64          # head dim
NPAIR = 4        # head pairs per core
KT = 8           # contraction tiles for projections
CH = 512         # query chunk width
NCHUNK = 4       # chunks per sequence
MT = 16          # key tiles (m) per sequence
ET = 8           # output-feature blocks
SCALE = HD ** -0.5
VW = HD + 1      # v columns per head incl. denominator ones-column
VFLAT = MT * NH * VW

_NC_CACHE = None


def _build():
    nc = bacc.Bacc("TRN2", target_bir_lowering=False, debug=False)

    xT = nc.dram_tensor("xT", [DI, N], F16, kind="ExternalInput").ap()
    wqkA = nc.dram_tensor("wqkA", [8, P, KT, P], F16, kind="ExternalInput").ap()
    wvA = nc.dram_tensor("wvA", [P, KT, 512], F16, kind="ExternalInput").ap()
    woT = nc.dram_tensor("woT", [512, DI], F16, kind="ExternalInput").ap()
    cstd = nc.dram_tensor("cst", [P, 129], F16, kind="ExternalInput").ap()
    outT = nc.dram_tensor("outT", [DI, N], F16, kind="ExternalOutput").ap()

    xT_r = xT.rearrange("(k p) n -> k p n", p=P)        # [8, 128, 2048]
    woT_r = woT.rearrange("(k p) e -> k p e", p=P)      # [4, 128, 1024]
    outT_r = outT.rearrange("(e p) n -> e p n", p=P)    # [8, 128, 2048]

    with tile.TileContext(nc) as tc, ExitStack() as persist:
        qk_pool = persist.enter_context(tc.tile_pool(name="qkp", bufs=8))
        va_pool = persist.enter_context(tc.tile_pool(name="vap", bufs=1))
        misc = persist.enter_context(tc.tile_pool(name="misc", bufs=1))
        wqk_pool = persist.enter_context(tc.tile_pool(name="wqk", bufs=2))
        xt_pool = persist.enter_context(tc.tile_pool(name="xt", bufs=8))
        wv_pool = persist.enter_context(tc.tile_pool(name="wv", bufs=1))
        wq_pool = persist.enter_context(tc.tile_pool(name="wq", bufs=4))
        wo_pool = persist.enter_context(tc.tile_pool(name="wo", bufs=4))
        exp_pool = persist.enter_context(tc.tile_pool(name="expp", bufs=30))
        ot_pool = persist.enter_context(tc.tile_pool(name="ot", bufs=8))
        osb_pool = persist.enter_context(tc.tile_pool(name="osb", bufs=4))
        stage_pool = persist.enter_context(tc.tile_pool(name="stg", bufs=3))
        den_pool = persist.enter_context(tc.tile_pool(name="den", bufs=8))
        rbc_pool = persist.enter_context(tc.tile_pool(name="rbc", bufs=2))
        sps_pool = persist.enter_context(
            tc.tile_pool(name="sps", bufs=2, space="PSUM"))
        oaug_pool = persist.enter_context(
            tc.tile_pool(name="oaug", bufs=2, space="PSUM"))
        aux_pool = persist.enter_context(
            tc.tile_pool(name="aux", bufs=2, space="PSUM"))

        # --- input DMAs, ordered so the pipeline lights up ASAP: the
        # first k-projection group needs wqk[4] + the chunk-0 columns of
        # every x k-tile; q needs wq0; v needs wv.
        wqk_first = wqk_pool.tile([P, KT, P], F16, tag="wqk")
        nc.sync.dma_start(wqk_first[:], wqkA[4])
        cst = misc.tile([P, 129], F16)
        xt = [xt_pool.tile([P, N], F16, name=f"xt{k}", tag="xt")
              for k in range(KT)]
        wq = [wq_pool.tile([P, KT, P], F16, name=f"wq{f}", tag="wq")
              for f in range(4)]
        for k in range(KT):
            nc.sync.dma_start(xt[k][:, 0:CH], xT_r[k][:, 0:CH])
        nc.sync.dma_start(wq[0][:], wqkA[0])
        nc.sync.dma_start(cst[:], cstd[:])
        wv = wv_pool.tile([P, KT, 512], F16)
        nc.sync.dma_start(wv[:], wvA[:])
        for cc in range(1, NCHUNK):
            csl = slice(cc * CH, (cc + 1) * CH)
            for k in range(KT):
                nc.sync.dma_start(xt[k][:, csl], xT_r[k][:, csl])
            if cc < 4:
                nc.sync.dma_start(wq[cc][:], wqkA[cc])
        wo = [wo_pool.tile([P, DI], F16, name=f"wo{kk}", tag="wo")
              for kk in range(NPAIR)]
        for kk in range(NPAIR):
            nc.sync.dma_start(wo[kk][:], woT_r[kk])

        qkT = [qk_pool.tile([P, N], F16, name=f"qkT{t}", tag="qkT")
               for t in range(8)]
        va_t = va_pool.tile([P, VFLAT + 64], F16)
        nc.vector.memset(va_t[:, VFLAT:VFLAT + 64], 0.0)
        v_aug = va_t[:, 0:VFLAT].rearrange("p (m h d) -> p m h d", h=NH, d=VW)
        nc.vector.tensor_copy(v_aug[:, :, :, HD:HD + 1],
                              cst[:, 0:1].to_broadcast((P, MT, NH, 1)))

        wqk_tiles = {4: wqk_first}

        # ---- emission helpers -------------------------------------------
        def emit_kproj_chunk(p, cc):
            f = 4 + p
            if f not in wqk_tiles:
                t = wqk_pool.tile([P, KT, P], F16, tag="wqk")
                nc.sync.dma_start(t[:], wqkA[f])
                wqk_tiles[f] = t
            wqk_f = wqk_tiles[f]
            csl = slice(cc * CH, (cc + 1) * CH)
            ps = aux_pool.tile([P, CH], F32, tag="aux", name=f"kp_{p}_{cc}")
            for k in range(KT):
                nc.tensor.matmul(ps[:], wqk_f[:, k, :], xt[k][:, csl],
                                 start=(k == 0), stop=(k == KT - 1))
            nc.vector.tensor_copy(qkT[4 + p][:, csl], ps[:])

        def emit_qproj(c, p):
            csl = slice(c * CH, (c + 1) * CH)
            ps = aux_pool.tile([P, CH], F32, tag="aux", name=f"qp_{c}_{p}")
            for k in range(KT):
                nc.tensor.matmul(ps[:], wq[p][:, k, :], xt[k][:, csl],
                                 start=(k == 0), stop=(k == KT - 1))
            nc.vector.tensor_copy(qkT[p][:, csl], ps[:])

        def emit_vproj(r):
            ps = aux_pool.tile([P, CH], F32, tag="aux", name=f"vp_{r}")
            for k in range(KT):
                nc.tensor.matmul(ps[:], xt[k][:, r * P:(r + 1) * P],
                                 wv[:, k, :],
                                 start=(k == 0), stop=(k == KT - 1))
            nc.vector.tensor_copy(v_aug[:, r, :, 0:HD],
                                  ps.rearrange("p (h d) -> p h d", d=HD))

        exp_map = {}   # (c, p) -> list of expP tiles

        def emit_S_pairs(c, p, ms):
            csl = slice(c * CH, (c + 1) * CH)
            qA = qkT[p][0:HD, csl]
            qB = qkT[p][HD:P, csl]
            kTl = qkT[4 + p]
            lst = exp_map.setdefault((c, p), [None] * MT)
            for m in ms:
                msl = slice(m * P, (m + 1) * P)
                s_ps = sps_pool.tile([P, 2 * CH], F32, tag="sps",
                                     name=f"sps_{c}_{p}_{m}")
                nc.tensor.matmul(s_ps[:, 0:CH], kTl[0:HD, msl], qA,
                                 start=True, stop=True)
                nc.tensor.matmul(s_ps[:, CH:2 * CH], kTl[HD:P, msl], qB,
                                 start=True, stop=True)
                expP = exp_pool.tile([P, 2 * CH], F16, tag="expp",
                                     name=f"expP_{c}_{p}_{m}")
                nc.scalar.activation(expP[:], s_ps[:],
                                     mybir.ActivationFunctionType.Exp,
                                     scale=SCALE)
                lst[m] = expP

        def emit_PV(c, p, fill):
            # fill: dict slot -> list of thunks emitted before that PV matmul.
            # The two heads' PV accumulations run as separate contiguous
            # sweeps (A then B) so each group's weight loads pull ahead into
            # the background buffer instead of serializing on the group
            # switch; the A sweep is exp-paced, the B sweep runs dense.
            oaugA = oaug_pool.tile([P, CH], F32, tag="oaug",
                                   name=f"oaugA_{c}_{p}")
            oaugB = oaug_pool.tile([P, CH], F32, tag="oaug",
                                   name=f"oaugB_{c}_{p}")
            expPs = exp_map.pop((c, p))
            for m in range(MT):
                for th in fill.get(m, ()):
                    th()
                vbase = (m * NH + 2 * p) * VW
                nc.tensor.matmul(oaugA[:, :], va_t[:, vbase:vbase + P],
                                 expPs[m][:, 0:CH],
                                 start=(m == 0), stop=(m == MT - 1))
            for m in range(MT):
                vbase = (m * NH + 2 * p + 1) * VW
                nc.tensor.matmul(oaugB[:, :], va_t[:, vbase:vbase + P],
                                 expPs[m][:, CH:2 * CH],
                                 start=(m == 0), stop=(m == MT - 1))
            # evacuate numerators + denominators (DVE only)
            o_sb = osb_pool.tile([P, CH], F32, tag="osb", name=f"osb_{c}_{p}")
            denA = den_pool.tile([1, CH], F16, tag="den", name=f"denA_{c}_{p}")
            denB = den_pool.tile([1, CH], F16, tag="den", name=f"denB_{c}_{p}")
            nc.vector.tensor_copy(o_sb[0:HD, :], oaugA[0:HD, :])
            nc.vector.tensor_copy(o_sb[HD:P, :], oaugB[0:HD, :])
            with nc.allow_low_precision(reason="softmax denom fp16"):
                nc.vector.tensor_copy(denA[:], oaugA[HD:HD + 1, :])
                nc.vector.tensor_copy(denB[:], oaugB[HD:HD + 1, :])
            return (c, p, o_sb, denA, denB)

        ot_map = {}

        def emit_norm(unit):
            c, p, o_sb, denA, denB = unit
            bc = aux_pool.tile([P, CH], F32, tag="aux", name=f"bc_{c}_{p}")
            nc.tensor.matmul(bc[0:HD, :], cst[0:1, 1:65], denA[:],
                             start=True, stop=True)
            nc.tensor.matmul(bc[HD:P, :], cst[0:1, 1:65], denB[:],
                             start=True, stop=True)
            rbc = rbc_pool.tile([P, CH], F32, tag="rbc", name=f"rbc_{c}_{p}")
            nc.vector.reciprocal_approx_fast(out=rbc[:], in_=bc[:])
            ot_p = ot_pool.tile([P, CH], F16, name=f"ot_{c}_{p}", tag="ot")
            nc.vector.tensor_tensor(ot_p[:], o_sb[:], rbc[:],
                                    mybir.AluOpType.mult)
            ot_map[(c, p)] = ot_p

        def emit_outproj_e(c, e):
            csl = slice(c * CH, (c + 1) * CH)
            pso = aux_pool.tile([P, CH], F32, tag="aux",
                                name=f"pso_{c}_{e}")
            for p in range(NPAIR):
                nc.tensor.matmul(pso[:], wo[p][:, e * P:(e + 1) * P],
                                 ot_map[(c, p)][:],
                                 start=(p == 0), stop=(p == NPAIR - 1))
            st = stage_pool.tile([P, CH], F16, tag="stg",
                                 name=f"st_{c}_{e}")
            with nc.allow_low_precision(reason="fp16 output partials"):
                nc.vector.tensor_copy(st[:], pso[:])
            nc.sync.dma_start(outT_r[e][:, csl], st[:])

        # ---- the pipeline -----------------------------------------------
        units = [(c, p) for c in range(NCHUNK) for p in range(NPAIR)]

        # prologue: unit (0,0) S phase with k-projection per chunk and the
        # first half of the v-projection woven in (all DMA-covered)
        for cc in range(NCHUNK):
            emit_kproj_chunk(0, cc)
            if cc == 0:
                emit_qproj(0, 0)
            emit_S_pairs(0, 0, range(4 * cc, 4 * cc + 4))
            if cc < 2:
                for r in range(4 * cc, 4 * cc + 4):
                    emit_vproj(r)
        # hoist first half of unit (0,1)'s S phase
        for cc in range(NCHUNK):
            emit_kproj_chunk(1, cc)
        emit_qproj(0, 1)
        emit_S_pairs(0, 1, range(0, 8))

        pend_norm = deque()
        normed = {c: 0 for c in range(NCHUNK)}
        pend_outproj = deque()

        for i, (c, p) in enumerate(units):
            nxt = units[i + 1] if i + 1 < len(units) else None
            nxt2 = units[i + 2] if i + 2 < len(units) else None

            fill = {}
            if (c, p) == (0, 0):
                # remaining v-projection row-blocks, just-in-time for PV
                for m in range(8, MT):
                    fill.setdefault(m, []).append(lambda r=m: emit_vproj(r))
            if nxt is not None:
                # second half of the next unit's S phase
                for j, m in enumerate(range(8, MT)):
                    fill.setdefault(j * 2, []).append(
                        lambda u=nxt, mm=m: emit_S_pairs(u[0], u[1], [mm]))
            if nxt2 is not None and nxt2[0] == 0:
                # k-projection for the unit after next, spread across slots
                for j in range(NCHUNK):
                    fill.setdefault(2 * j + 1, []).append(
                        lambda p2=nxt2[1], cc=j: emit_kproj_chunk(p2, cc))
            if nxt2 is not None:
                # q-projection + first S pairs of the unit after next, woven
                # into the late slots so the scalar engine never drains
                # across the unit boundary (the PV-B sweep has no exps)
                c2, p2 = nxt2
                fill.setdefault(8, []).append(
                    lambda: emit_qproj(c2, p2))
                for j, m in enumerate(range(0, 4)):
                    fill.setdefault(9 + 2 * j, []).append(
                        lambda u=nxt2, mm=m: emit_S_pairs(u[0], u[1], [mm]))
            # spread pending out-projection blocks across late slots
            for j in range(8, MT):
                if pend_outproj:
                    th = pend_outproj.popleft()
                    fill.setdefault(j, []).append(th)

            unit = emit_PV(c, p, fill)
            pend_norm.append(unit)

            # post-block: rest of nxt2's first S half, then lagged norm
            if nxt2 is not None:
                emit_S_pairs(nxt2[0], nxt2[1], range(4, 8))
            if len(pend_norm) > 1:
                u = pend_norm.popleft()
                emit_norm(u)
                normed[u[0]] += 1
                if normed[u[0]] == NPAIR:
                    cc = u[0]
                    for e in range(ET):
                        pend_outproj.append(
                            lambda c2=cc, ee=e: emit_outproj_e(c2, ee))

        # tail: drain norms + remaining out-projections
        while pend_norm:
            u = pend_norm.popleft()
            emit_norm(u)
            normed[u[0]] += 1
        for th in pend_outproj:
            th()
        for e in range(ET):
            emit_outproj_e(NCHUNK - 1, e)

    nc.compile()
    return nc


def _get_nc():
    global _NC_CACHE
    if _NC_CACHE is None:
        _NC_CACHE = _build()
    return _NC_CACHE


def _make_in_maps(x, w_qkv, w_out):
    cst = np.zeros((P, 129), dtype=np.float16)
    cst[:, 0] = 1.0
    cst[0, 1:65] = 1.0
    cst[1, 65:129] = 1.0
    per_g = []
    for g in range(2):
        qk_g = np.concatenate([w_qkv[g * 512:(g + 1) * 512],
                               w_qkv[DI + g * 512:DI + (g + 1) * 512]], axis=0)
        wqkT = np.ascontiguousarray(qk_g.T)               # [1024 d, 1024 f]
        wqkA = np.ascontiguousarray(
            wqkT.reshape(KT, P, 8, P).transpose(2, 1, 0, 3).astype(np.float16))
        v_g = w_qkv[2 * DI + g * 512:2 * DI + (g + 1) * 512]
        wvT = np.ascontiguousarray(v_g.T)                 # [1024 d, 512 f]
        wvA = np.ascontiguousarray(
            wvT.reshape(KT, P, 512).transpose(1, 0, 2).astype(np.float16))
        woTg = np.ascontiguousarray(
            w_out[:, g * 512:(g + 1) * 512].T.astype(np.float16))
        per_g.append((wqkA, wvA, woTg))

    in_maps = []
    for c in range(8):
        b, g = c // 2, c % 2
        wqkA, wvA, woTg = per_g[g]
        in_maps.append({
            "xT": np.ascontiguousarray(x[b].T.astype(np.float16)),
            "wqkA": wqkA,
            "wvA": wvA,
            "woT": woTg,
            "cst": cst,
        })
    return in_maps


def kernel(x, w_qkv, w_out, b_out):
    x = np.asarray(x, dtype=np.float32)
    w_qkv = np.asarray(w_qkv, dtype=np.float32)
    w_out = np.asarray(w_out, dtype=np.float32)
    b_out = np.asarray(b_out, dtype=np.float32)
    B = x.shape[0]

    in_maps = _make_in_maps(x, w_qkv, w_out)
    nc = _get_nc()
    res = run_bass_kernel_spmd(nc, in_maps, core_ids=list(range(8)))
    parts = [r["outT"] for r in res.results]
    out = np.empty((B, N, DI), dtype=np.float32)
    for b in range(B):
        out[b] = (parts[2 * b].astype(np.float32)
                  + parts[2 * b + 1].astype(np.float32)).T + b_out
    return out


# revision 18
# speedup vs baseline: 1.1627x; 1.0053x over previous
"""Multi-head attention Trainium2 kernel (B=4, N=2048, D=1024, H=16).

Sharding: 8 cores = 4 batches x 2 head-groups (8 heads each), zero
collectives. Each core:
  - all projections in fp16, interleaved into the attention pipeline so
    the scalar engine (exp) starts ~10us in and stays saturated:
    x arrives as 512-column slices; the first head-pair's k-projection and
    q-projection run as the slices land, its S matmuls follow immediately,
    and the v-projection row-blocks are emitted just-in-time before the
    PV matmul that consumes them
  - q,k kept transposed [feat, seq]; v row-layout, augmented with a ones
    column so the PV matmul emits the softmax denominator for free
  - attention per head-pair x 512-query chunk: S matmuls packed two heads
    per pass via disjoint PE row groups into one [128,1024] PSUM tile,
    one wide exp on ACT (scale=1/8, fp16 out), PV accumulation with
    128-column stationary windows (fast-weight-load path)
  - software pipeline: each unit's S phase is split 8/8 around the
    previous unit's PV loop (exp pool holds ~24 tiles) so the scalar
    engine never starves; projection/out-projection/normalization work
    fills the PE slack inside the ACT-paced PV loops
  - normalization: one 64-wide ones matmul per head broadcasts the fp16
    denominator across partitions, one reciprocal_approx_fast and one
    multiply produce the normalized [128, 512] fp16 tile per unit
  - out-projection partial [1024,2048] per chunk, evacuated via DVE
Host sums the two head-group partials per batch and adds bias.
"""
from collections import deque
from contextlib import ExitStack

import numpy as np

import concourse.mybir as mybir
import concourse.tile as tile
from concourse import bacc
from concourse.bass_utils import run_bass_kernel_spmd

F32 = mybir.dt.float32
F16 = mybir.dt.float16

P = 128
N = 2048         # sequence length
DI = 1024        # model dim
NH = 8           # heads per core
HD = 64          # head dim
NPAIR = 4        # head pairs per core
KT = 8           # contraction tiles for projections
CH = 512         # query chunk width
NCHUNK = 4       # chunks per sequence
MT = 16          # key tiles (m) per sequence
ET = 8           # output-feature blocks
SCALE = HD ** -0.5
VW = HD + 1      # v columns per head incl. denominator ones-column
VFLAT = MT * NH * VW

_NC_CACHE = None


def _build():
    nc = bacc.Bacc("TRN2", target_bir_lowering=False, debug=False)

    xT = nc.dram_tensor("xT", [DI, N], F16, kind="ExternalInput").ap()
    wqkA = nc.dram_tensor("wqkA", [8, P, KT, P], F16, kind="ExternalInput").ap()
    wvA = nc.dram_tensor("wvA", [P, KT, 512], F16, kind="ExternalInput").ap()
    woT = nc.dram_tensor("woT", [512, DI], F16, kind="ExternalInput").ap()
    cstd = nc.dram_tensor("cst", [P, 129], F16, kind="ExternalInput").ap()
    outT = nc.dram_tensor("outT", [DI, N], F16, kind="ExternalOutput").ap()

    xT_r = xT.rearrange("(k p) n -> k p n", p=P)        # [8, 128, 2048]
    woT_r = woT.rearrange("(k p) e -> k p e", p=P)      # [4, 128, 1024]
    outT_r = outT.rearrange("(e p) n -> e p n", p=P)    # [8, 128, 2048]

    with tile.TileContext(nc) as tc, ExitStack() as persist:
        qk_pool = persist.enter_context(tc.tile_pool(name="qkp", bufs=8))
        va_pool = persist.enter_context(tc.tile_pool(name="vap", bufs=1))
        misc = persist.enter_context(tc.tile_pool(name="misc", bufs=1))
        wqk_pool = persist.enter_context(tc.tile_pool(name="wqk", bufs=2))
        xt_pool = persist.enter_context(tc.tile_pool(name="xt", bufs=8))
        wv_pool = persist.enter_context(tc.tile_pool(name="wv", bufs=1))
        wq_pool = persist.enter_context(tc.tile_pool(name="wq", bufs=4))
        wo_pool = persist.enter_context(tc.tile_pool(name="wo", bufs=4))
        exp_pool = persist.enter_context(tc.tile_pool(name="expp", bufs=30))
        ot_pool = persist.enter_context(tc.tile_pool(name="ot", bufs=8))
        osb_pool = persist.enter_context(tc.tile_pool(name="osb", bufs=4))
        stage_pool = persist.enter_context(tc.tile_pool(name="stg", bufs=3))
        den_pool = persist.enter_context(tc.tile_pool(name="den", bufs=8))
        rbc_pool = persist.enter_context(tc.tile_pool(name="rbc", bufs=2))
        sps_pool = persist.enter_context(
            tc.tile_pool(name="sps", bufs=2, space="PSUM"))
        oaug_pool = persist.enter_context(
            tc.tile_pool(name="oaug", bufs=2, space="PSUM"))
        aux_pool = persist.enter_context(
            tc.tile_pool(name="aux", bufs=2, space="PSUM"))

        # --- input DMAs, ordered so the pipeline lights up ASAP: the
        # first k-projection group needs wqk[4] + the chunk-0 columns of
        # every x k-tile; q needs wq0; v needs wv.
        wqk_first = wqk_pool.tile([P, KT, P], F16, tag="wqk")
        nc.sync.dma_start(wqk_first[:], wqkA[4])
        cst = misc.tile([P, 129], F16)
        xt = [xt_pool.tile([P, N], F16, name=f"xt{k}", tag="xt")
              for k in range(KT)]
        wq = [wq_pool.tile([P, KT, P], F16, name=f"wq{f}", tag="wq")
              for f in range(4)]
        for k in range(KT):
            nc.sync.dma_start(xt[k][:, 0:CH], xT_r[k][:, 0:CH])
        nc.sync.dma_start(wq[0][:], wqkA[0])
        nc.sync.dma_start(cst[:], cstd[:])
        wv = wv_pool.tile([P, KT, 512], F16)
        nc.sync.dma_start(wv[:], wvA[:])
        for cc in range(1, NCHUNK):
            csl = slice(cc * CH, (cc + 1) * CH)
            for k in range(KT):
                nc.sync.dma_start(xt[k][:, csl], xT_r[k][:, csl])
            if cc < 4:
                nc.sync.dma_start(wq[cc][:], wqkA[cc])
        wo = [wo_pool.tile([P, DI], F16, name=f"wo{kk}", tag="wo")
              for kk in range(NPAIR)]
        for kk in range(NPAIR):
            nc.sync.dma_start(wo[kk][:], woT_r[kk])

        qkT = [qk_pool.tile([P, N], F16, name=f"qkT{t}", tag="qkT")
               for t in range(8)]
        va_t = va_pool.tile([P, VFLAT + 64], F16)
        nc.vector.memset(va_t[:, VFLAT:VFLAT + 64], 0.0)
        v_aug = va_t[:, 0:VFLAT].rearrange("p (m h d) -> p m h d", h=NH, d=VW)
        nc.vector.tensor_copy(v_aug[:, :, :, HD:HD + 1],
                              cst[:, 0:1].to_broadcast((P, MT, NH, 1)))

        wqk_tiles = {4: wqk_first}

        # ---- emission helpers -------------------------------------------
        def emit_kproj_chunk(p, cc):
            f = 4 + p
            if f not in wqk_tiles:
                t = wqk_pool.tile([P, KT, P], F16, tag="wqk")
                nc.sync.dma_start(t[:], wqkA[f])
                wqk_tiles[f] = t
            wqk_f = wqk_tiles[f]
            csl = slice(cc * CH, (cc + 1) * CH)
            ps = aux_pool.tile([P, CH], F32, tag="aux", name=f"kp_{p}_{cc}")
            for k in range(KT):
                nc.tensor.matmul(ps[:], wqk_f[:, k, :], xt[k][:, csl],
                                 start=(k == 0), stop=(k == KT - 1))
            nc.vector.tensor_copy(qkT[4 + p][:, csl], ps[:])

        def emit_qproj(c, p):
            csl = slice(c * CH, (c + 1) * CH)
            ps = aux_pool.tile([P, CH], F32, tag="aux", name=f"qp_{c}_{p}")
            for k in range(KT):
                nc.tensor.matmul(ps[:], wq[p][:, k, :], xt[k][:, csl],
                                 start=(k == 0), stop=(k == KT - 1))
            nc.vector.tensor_copy(qkT[p][:, csl], ps[:])

        def emit_vproj(r):
            ps = aux_pool.tile([P, CH], F32, tag="aux", name=f"vp_{r}")
            for k in range(KT):
                nc.tensor.matmul(ps[:], xt[k][:, r * P:(r + 1) * P],
                                 wv[:, k, :],
                                 start=(k == 0), stop=(k == KT - 1))
            nc.vector.tensor_copy(v_aug[:, r, :, 0:HD],
                                  ps.rearrange("p (h d) -> p h d", d=HD))

        exp_map = {}   # (c, p) -> list of expP tiles

        def emit_S_pairs(c, p, ms):
            csl = slice(c * CH, (c + 1) * CH)
            qA = qkT[p][0:HD, csl]
            qB = qkT[p][HD:P, csl]
            kTl = qkT[4 + p]
            lst = exp_map.setdefault((c, p), [None] * MT)
            for m in ms:
                msl = slice(m * P, (m + 1) * P)
                s_ps = sps_pool.tile([P, 2 * CH], F32, tag="sps",
                                     name=f"sps_{c}_{p}_{m}")
                nc.tensor.matmul(s_ps[:, 0:CH], kTl[0:HD, msl], qA,
                                 start=True, stop=True)
                nc.tensor.matmul(s_ps[:, CH:2 * CH], kTl[HD:P, msl], qB,
                                 start=True, stop=True)
                expP = exp_pool.tile([P, 2 * CH], F16, tag="expp",
                                     name=f"expP_{c}_{p}_{m}")
                nc.scalar.activation(expP[:], s_ps[:],
                                     mybir.ActivationFunctionType.Exp,
                                     scale=SCALE)
                lst[m] = expP

        def emit_PV(c, p, fill, fillB=None):
            # fill: dict slot -> list of thunks emitted before that PV matmul.
            # The two heads' PV accumulations run as separate contiguous
            # sweeps (A then B) so each group's weight loads pull ahead into
            # the background buffer instead of serializing on the group
            # switch; the A sweep is exp-paced, the B sweep runs dense.
            oaugA = oaug_pool.tile([P, CH], F32, tag="oaug",
                                   name=f"oaugA_{c}_{p}")
            oaugB = oaug_pool.tile([P, CH], F32, tag="oaug",
                                   name=f"oaugB_{c}_{p}")
            expPs = exp_map.pop((c, p))
            for m in range(MT):
                for th in fill.get(m, ()):
                    th()
                vbase = (m * NH + 2 * p) * VW
                nc.tensor.matmul(oaugA[:, :], va_t[:, vbase:vbase + P],
                                 expPs[m][:, 0:CH],
                                 start=(m == 0), stop=(m == MT - 1))
            for m in range(MT):
                if fillB:
                    for th in fillB.get(m, ()):
                        th()
                vbase = (m * NH + 2 * p + 1) * VW
                nc.tensor.matmul(oaugB[:, :], va_t[:, vbase:vbase + P],
                                 expPs[m][:, CH:2 * CH],
                                 start=(m == 0), stop=(m == MT - 1))
            # evacuate numerators + denominators (DVE only)
            o_sb = osb_pool.tile([P, CH], F32, tag="osb", name=f"osb_{c}_{p}")
            denA = den_pool.tile([1, CH], F16, tag="den", name=f"denA_{c}_{p}")
            denB = den_pool.tile([1, CH], F16, tag="den", name=f"denB_{c}_{p}")
            nc.vector.tensor_copy(o_sb[0:HD, :], oaugA[0:HD, :])
            nc.vector.tensor_copy(o_sb[HD:P, :], oaugB[0:HD, :])
            with nc.allow_low_precision(reason="softmax denom fp16"):
                nc.vector.tensor_copy(denA[:], oaugA[HD:HD + 1, :])
                nc.vector.tensor_copy(denB[:], oaugB[HD:HD + 1, :])
            return (c, p, o_sb, denA, denB)

        ot_map = {}

        def emit_norm(unit):
            c, p, o_sb, denA, denB = unit
            bc = aux_pool.tile([P, CH], F32, tag="aux", name=f"bc_{c}_{p}")
            nc.tensor.matmul(bc[0:HD, :], cst[0:1, 1:65], denA[:],
                             start=True, stop=True)
            nc.tensor.matmul(bc[HD:P, :], cst[0:1, 1:65], denB[:],
                             start=True, stop=True)
            rbc = rbc_pool.tile([P, CH], F32, tag="rbc", name=f"rbc_{c}_{p}")
            nc.vector.reciprocal_approx_fast(out=rbc[:], in_=bc[:])
            ot_p = ot_pool.tile([P, CH], F16, name=f"ot_{c}_{p}", tag="ot")
            nc.vector.tensor_tensor(ot_p[:], o_sb[:], rbc[:],
                                    mybir.AluOpType.mult)
            ot_map[(c, p)] = ot_p

        def emit_outproj_e(c, e):
            csl = slice(c * CH, (c + 1) * CH)
            pso = aux_pool.tile([P, CH], F32, tag="aux",
                                name=f"pso_{c}_{e}")
            for p in range(NPAIR):
                nc.tensor.matmul(pso[:], wo[p][:, e * P:(e + 1) * P],
                                 ot_map[(c, p)][:],
                                 start=(p == 0), stop=(p == NPAIR - 1))
            st = stage_pool.tile([P, CH], F16, tag="stg",
                                 name=f"st_{c}_{e}")
            with nc.allow_low_precision(reason="fp16 output partials"):
                nc.vector.tensor_copy(st[:], pso[:])
            nc.sync.dma_start(outT_r[e][:, csl], st[:])

        # ---- the pipeline -----------------------------------------------
        units = [(c, p) for c in range(NCHUNK) for p in range(NPAIR)]

        # prologue: unit (0,0) S phase with k-projection per chunk and the
        # first half of the v-projection woven in (all DMA-covered)
        for cc in range(NCHUNK):
            emit_kproj_chunk(0, cc)
            if cc == 0:
                emit_qproj(0, 0)
            emit_S_pairs(0, 0, range(4 * cc, 4 * cc + 4))
            if cc < 2:
                for r in range(4 * cc, 4 * cc + 4):
                    emit_vproj(r)
        # hoist first half of unit (0,1)'s S phase
        for cc in range(NCHUNK):
            emit_kproj_chunk(1, cc)
        emit_qproj(0, 1)
        emit_S_pairs(0, 1, range(0, 8))

        pend_norm = deque()
        normed = {c: 0 for c in range(NCHUNK)}
        pend_outproj = deque()

        for i, (c, p) in enumerate(units):
            nxt = units[i + 1] if i + 1 < len(units) else None
            nxt2 = units[i + 2] if i + 2 < len(units) else None

            fill = {}
            if (c, p) == (0, 0):
                # remaining v-projection row-blocks, just-in-time for PV
                for m in range(8, MT):
                    fill.setdefault(m, []).append(lambda r=m: emit_vproj(r))
            if nxt is not None:
                # second half of the next unit's S phase
                for j, m in enumerate(range(8, MT)):
                    fill.setdefault(j * 2, []).append(
                        lambda u=nxt, mm=m: emit_S_pairs(u[0], u[1], [mm]))
            if nxt2 is not None and nxt2[0] == 0:
                # k-projection for the unit after next, spread across slots
                for j in range(NCHUNK):
                    fill.setdefault(2 * j + 1, []).append(
                        lambda p2=nxt2[1], cc=j: emit_kproj_chunk(p2, cc))
            if nxt2 is not None:
                # q-projection + first S pairs of the unit after next, woven
                # into the late slots so the scalar engine never drains
                # across the unit boundary (the PV-B sweep has no exps)
                c2, p2 = nxt2
                fill.setdefault(8, []).append(
                    lambda: emit_qproj(c2, p2))
                for j, m in enumerate(range(0, 4)):
                    fill.setdefault(9 + 2 * j, []).append(
                        lambda u=nxt2, mm=m: emit_S_pairs(u[0], u[1], [mm]))
            # spread pending out-projection blocks across late slots
            for j in range(8, MT):
                if pend_outproj:
                    th = pend_outproj.popleft()
                    fill.setdefault(j, []).append(th)

            fillB = {}
            if nxt2 is not None:
                # rest of nxt2's first S half, woven into the B sweep so the
                # scalar engine keeps working through it
                for j, m in enumerate(range(4, 8)):
                    fillB[2 + 4 * j] = [
                        lambda u=nxt2, mm=m: emit_S_pairs(u[0], u[1], [mm])]

            unit = emit_PV(c, p, fill, fillB)
            pend_norm.append(unit)
            if len(pend_norm) > 1:
                u = pend_norm.popleft()
                emit_norm(u)
                normed[u[0]] += 1
                if normed[u[0]] == NPAIR:
                    cc = u[0]
                    for e in range(ET):
                        pend_outproj.append(
                            lambda c2=cc, ee=e: emit_outproj_e(c2, ee))

        # tail: drain norms + remaining out-projections
        while pend_norm:
            u = pend_norm.popleft()
            emit_norm(u)
            normed[u[0]] += 1
        for th in pend_outproj:
            th()
        for e in range(ET):
            emit_outproj_e(NCHUNK - 1, e)

    nc.compile()
    return nc


def _get_nc():
    global _NC_CACHE
    if _NC_CACHE is None:
        _NC_CACHE = _build()
    return _NC_CACHE


def _make_in_maps(x, w_qkv, w_out):
    cst = np.zeros((P, 129), dtype=np.float16)
    cst[:, 0] = 1.0
    cst[0, 1:65] = 1.0
    cst[1, 65:129] = 1.0
    per_g = []
    for g in range(2):
        qk_g = np.concatenate([w_qkv[g * 512:(g + 1) * 512],
                               w_qkv[DI + g * 512:DI + (g + 1) * 512]], axis=0)
        wqkT = np.ascontiguousarray(qk_g.T)               # [1024 d, 1024 f]
        wqkA = np.ascontiguousarray(
            wqkT.reshape(KT, P, 8, P).transpose(2, 1, 0, 3).astype(np.float16))
        v_g = w_qkv[2 * DI + g * 512:2 * DI + (g + 1) * 512]
        wvT = np.ascontiguousarray(v_g.T)                 # [1024 d, 512 f]
        wvA = np.ascontiguousarray(
            wvT.reshape(KT, P, 512).transpose(1, 0, 2).astype(np.float16))
        woTg = np.ascontiguousarray(
            w_out[:, g * 512:(g + 1) * 512].T.astype(np.float16))
        per_g.append((wqkA, wvA, woTg))

    in_maps = []
    for c in range(8):
        b, g = c // 2, c % 2
        wqkA, wvA, woTg = per_g[g]
        in_maps.append({
            "xT": np.ascontiguousarray(x[b].T.astype(np.float16)),
            "wqkA": wqkA,
            "wvA": wvA,
            "woT": woTg,
            "cst": cst,
        })
    return in_maps


def kernel(x, w_qkv, w_out, b_out):
    x = np.asarray(x, dtype=np.float32)
    w_qkv = np.asarray(w_qkv, dtype=np.float32)
    w_out = np.asarray(w_out, dtype=np.float32)
    b_out = np.asarray(b_out, dtype=np.float32)
    B = x.shape[0]

    in_maps = _make_in_maps(x, w_qkv, w_out)
    nc = _get_nc()
    res = run_bass_kernel_spmd(nc, in_maps, core_ids=list(range(8)))
    parts = [r["outT"] for r in res.results]
    out = np.empty((B, N, DI), dtype=np.float32)
    for b in range(B):
        out[b] = (parts[2 * b].astype(np.float32)
                  + parts[2 * b + 1].astype(np.float32)).T + b_out
    return out
